# revision 16
# baseline (speedup 1.0000x reference)
"""Trainium2 Bass kernel for the entity-assignment loss.

Math: per sample b, C[i,j] = mean_d (yt[b,i,d]-yp[b,j,d])^2.
loss = mean_b ( min_perm sum_i C[i, perm(i)] / 8 ).

Since each permutation uses every row i and every column j exactly once,
  sum_i C[i, perm(i)] = (nt + np - 2 * sum_i dot(i, perm(i))) / 64
with nt = sum_i |yt_i|^2, np = sum_j |yp_j|^2 (per-sample constants).
So min over perms only needs MAX over perms of the dot sum, computed with a
2^8 bitmask DP whose bit-i update is a perfectly strided access pattern.

Engine budget (measured): the fused add+max (scalar_tensor_tensor) only
runs on the Vector engine at 1x (~262ns issue floor per 128-mask update),
so the 96-update DP is DVE-bound; neuronxcc rejects tensor ops on GpSimd.
This version: inputs pre-cast to f16 on the host (halves HBM traffic; the
dot products used f16 anyway), two DMA rings (Sync+Scalar), fused per-chunk
norms via one Square+accum activation, k0-init on the Scalar engine, and
Scalar-engine candidate adds (bias=[p,1] G column) for DP rows i>=IMIN so
DVE accumulates those rows with cheaper 2x tensor-tensor maxes. Candidate
buffers use a disjoint 256-wide region per row and double-buffer by column
parity: the bit-i half-spaces of different rows overlap, so sharing one
buffer corrupts earlier candidates before they are consumed.

Sharding: pure data parallelism, 256 samples per core across 8 cores; the
final mean is taken on the host from per-sample partial results.
"""

import os
import sys

if "/opt/trn_rl_repo" not in sys.path:
    sys.path.insert(0, "/opt/trn_rl_repo")

import numpy as np

B, N, D = 2048, 8, 64
N_CORES = 8
B_LOC = B // N_CORES        # 256 samples per core
NEG = -60000.0

# tunables
ACAND = os.environ.get("K_ACAND", "1") == "1"   # Act computes cands for i>=IMIN
IMIN = int(os.environ.get("K_IMIN", "5"))
K0ACT = os.environ.get("K_K0ACT", "1") == "1"   # k0 init on Act engine

TRACE = False
_CACHE = {}


def _build():
    import concourse.bacc as bacc
    import concourse.mybir as mybir
    from concourse.tile import TileContext

    f32 = mybir.dt.float32
    f16 = mybir.dt.float16
    Alu = mybir.AluOpType
    Act = mybir.ActivationFunctionType

    nc = bacc.Bacc("TRN2", target_bir_lowering=False, debug=False)
    yt_d = nc.declare_dram_parameter("yt", [B_LOC, N * D], f16, isOutput=False)
    yp_d = nc.declare_dram_parameter("yp", [B_LOC, N * D], f16, isOutput=False)
    out_d = nc.declare_dram_parameter("out", [128, 2], f32, isOutput=True)

    with TileContext(nc) as tc:
        with (
            tc.tile_pool(name="io", bufs=1) as io,
            tc.tile_pool(name="work", bufs=1) as work,
            tc.tile_pool(name="res", bufs=1) as res,
        ):
            loss_t = res.tile([128, 2], f32, tag="loss")
            dummy = res.tile([128, 1], f32, tag="dummy")

            ytp16, G32, s_all = [], [], []
            dpo, dpn, cand, dmax = [], [], [], []
            acb = []
            for c in range(2):
                ytp16.append(work.tile([128, 2 * N * D], f16, name=f"ytp16_{c}",
                                       tag=f"ytp16_{c}"))
                G32.append(res.tile([128, N * N], f32, name=f"G32_{c}",
                                    tag=f"G32_{c}"))
                s_all.append(res.tile([128, 1], f32, name=f"sall_{c}",
                                      tag=f"sall_{c}"))
                dpo.append(res.tile([128, 256], f16, name=f"dpo_{c}",
                                    tag=f"dpo_{c}"))
                dpn.append(res.tile([128, 256], f16, name=f"dpn_{c}",
                                    tag=f"dpn_{c}"))
                cand.append(res.tile([128, N], f16, name=f"cand_{c}",
                                     tag=f"cand_{c}"))
                dmax.append(res.tile([128, 1], f16, name=f"dmax_{c}",
                                     tag=f"dmax_{c}"))
                acb.append([res.tile([128, (N - IMIN) * 256], f16,
                                     name=f"acb_{c}0", tag=f"acb_{c}0"),
                            res.tile([128, (N - IMIN) * 256], f16,
                                     name=f"acb_{c}1", tag=f"acb_{c}1")])

            # input DMAs on two hardware rings (inputs pre-cast to f16 on
            # the host, halving HBM traffic): yt via Sync, yp via Scalar,
            # chunk 0 first.
            nc.sync.dma_start(out=ytp16[0][:, 0:N * D], in_=yt_d[0:128, :])
            nc.sync.dma_start(out=ytp16[1][:, 0:N * D], in_=yt_d[128:256, :])
            nc.scalar.dma_start(out=ytp16[0][:, N * D:], in_=yp_d[0:128, :])
            nc.scalar.dma_start(out=ytp16[1][:, N * D:], in_=yp_d[128:256, :])

            # DP-state memsets on GpSimd (idle, overlapped with DMA)
            for c in range(2):
                nc.gpsimd.memset(dpo[c][:, :], NEG)
                nc.gpsimd.memset(dpn[c][:, :], NEG)

            # norms: one Square+accum per chunk over the merged [yt|yp] tile
            sq = work.tile([128, 2 * N * D], f32, tag="sq")
            for c in range(2):
                nc.scalar.activation(out=sq[:, :], in_=ytp16[c][:, :],
                                     func=Act.Square,
                                     accum_out=s_all[c][:, :])

            # ---- G-stage on DVE ----
            def g_stage(c, jsplit=False):
                yt16 = ytp16[c][:, 0:N * D]
                yp16 = ytp16[c][:, N * D:2 * N * D]
                yt_b = yt16.rearrange("p (i d) -> p i d", d=D).unsqueeze(2) \
                    .broadcast_to([128, N, N, D])
                yp_b = yp16.rearrange("p (j d) -> p j d", d=D).unsqueeze(1) \
                    .broadcast_to([128, N, N, D])
                prod = work.tile([128, N * N * D], f16, tag=f"prod_{c}")
                pview = prod.rearrange("p (i j d) -> p i j d", j=N, d=D)
                if jsplit:
                    nc.vector.tensor_tensor(
                        out=pview[:, :, 0:N // 2, :],
                        in0=yt_b[:, :, 0:N // 2, :],
                        in1=yp_b[:, :, 0:N // 2, :], op=Alu.mult)
                    nc.vector.tensor_tensor(
                        out=pview[:, :, N // 2:, :],
                        in0=yt_b[:, :, N // 2:, :],
                        in1=yp_b[:, :, N // 2:, :], op=Alu.mult)
                else:
                    nc.vector.tensor_tensor(
                        out=pview, in0=yt_b, in1=yp_b, op=Alu.mult)
                pv = prod.rearrange("p (q d) -> p q d", d=D)
                half = work.tile([128, N * N * D // 2], f16, tag=f"half_{c}")
                hv = half.rearrange("p (q d) -> p q d", d=D // 2)
                nc.vector.tensor_tensor(
                    out=hv, in0=pv[:, :, 0:D // 2], in1=pv[:, :, D // 2:D],
                    op=Alu.add)
                quart = work.tile([128, N * N * D // 4], f16, tag=f"quart_{c}")
                qv = quart.rearrange("p (q d) -> p q d", d=D // 4)
                nc.vector.tensor_tensor(
                    out=qv, in0=hv[:, :, 0:D // 4], in1=hv[:, :, D // 4:D // 2],
                    op=Alu.add)
                eighth = work.tile([128, N * N * D // 8], f16, tag=f"eighth_{c}")
                ev = eighth.rearrange("p (q d) -> p q d", d=D // 8)
                nc.vector.tensor_tensor(
                    out=ev, in0=qv[:, :, 0:D // 8], in1=qv[:, :, D // 8:D // 4],
                    op=Alu.add)
                nc.vector.tensor_reduce(
                    out=G32[c][:, :], in_=ev, axis=mybir.AxisListType.X,
                    op=Alu.add)

            def k0_init(c):
                for i in range(0, N, 2):
                    ci = 2 ** i
                    tgt = dpo[c][:, ci:2 * ci + 1:ci]
                    gsrc = G32[c][:, i * N:(i + 2) * N:N]
                    if K0ACT:
                        nc.scalar.activation(out=tgt, in_=gsrc,
                                             func=Act.Identity)
                    else:
                        nc.vector.tensor_copy(tgt, gsrc)

            def act_cands(k, cs):
                # Act precomputes cands for rows i>=IMIN of column k;
                # acb double-buffered by column parity to avoid any WAR
                # overlap with the previous column's consumers
                for i in range(IMIN, N):
                    ci = 2 ** i
                    col = i * N + k
                    for c in cs:
                        old = bufs[c][(k + 1) % 2]
                        vo = old.rearrange("p (a b x) -> p a b x", b=2, x=ci)
                        lo = (i - IMIN) * 256
                        va = acb[c][k % 2][:, lo:lo + 256].rearrange(
                            "p (a b x) -> p a b x", b=2, x=ci)
                        nc.scalar.activation(out=va[:, :, 1, :],
                                             in_=vo[:, :, 0, :],
                                             func=Act.Identity,
                                             bias=G32[c][:, col:col + 1])

            bufs = [[dpo[0], dpn[0]], [dpo[1], dpn[1]]]
            g_stage(0, jsplit=True)
            # during G1 on DVE, Act initializes chunk-0 DP and its col-1 cands
            k0_init(0)
            if ACAND:
                act_cands(1, [0])
            g_stage(1)
            k0_init(1)
            if ACAND:
                act_cands(1, [1])

            # ---- DP columns k=1..6 on DVE; Act feeds cands for i>=IMIN ----
            for k in range(1, N - 1):
                old = [bufs[c][(k + 1) % 2] for c in range(2)]
                new = [bufs[c][k % 2] for c in range(2)]
                for c in range(2):
                    for i in range(N if not ACAND else IMIN):
                        ci = 2 ** i
                        col = i * N + k
                        vo = old[c].rearrange("p (a b x) -> p a b x", b=2, x=ci)
                        vn = new[c].rearrange("p (a b x) -> p a b x", b=2, x=ci)
                        nc.vector.scalar_tensor_tensor(
                            out=vn[:, :, 1, :], in0=vo[:, :, 0, :],
                            scalar=G32[c][:, col:col + 1],
                            in1=vn[:, :, 1, :], op0=Alu.add, op1=Alu.max)
                if ACAND:
                    for c in range(2):
                        for i in range(IMIN, N):
                            ci = 2 ** i
                            vn = new[c].rearrange("p (a b x) -> p a b x",
                                                  b=2, x=ci)
                            lo = (i - IMIN) * 256
                            va = acb[c][k % 2][:, lo:lo + 256].rearrange(
                                "p (a b x) -> p a b x", b=2, x=ci)
                            nc.vector.tensor_tensor(out=vn[:, :, 1, :],
                                                    in0=va[:, :, 1, :],
                                                    in1=vn[:, :, 1, :],
                                                    op=Alu.max)
                    if k < N - 2:
                        act_cands(k + 1, [0, 1])

            # ---- k7: gather final candidates, reduce, combine ----
            k = N - 1
            for c in range(2):
                old = bufs[c][(k + 1) % 2]
                for i in range(0, N, 2):
                    ci = 2 ** i
                    src = old[:, 255 - 2 * ci:256 - ci:ci]
                    cv = cand[c][:, i:i + 2]
                    try:
                        gsrc = G32[c][:, (i + 1) * N + k::-N][:, 0:2]
                        nc.vector.tensor_tensor(out=cv, in0=src, in1=gsrc,
                                                op=Alu.add)
                    except Exception:
                        for t, row in enumerate((i + 1, i)):
                            nc.vector.tensor_tensor(
                                out=cv[:, t:t + 1], in0=src[:, t:t + 1],
                                in1=G32[c][:, row * N + k:row * N + k + 1],
                                op=Alu.add)
                nc.vector.tensor_reduce(out=dmax[c][:, :], in_=cand[c][:, :],
                                        axis=mybir.AxisListType.X, op=Alu.max)
                nc.vector.scalar_tensor_tensor(
                    out=loss_t[:, c:c + 1], in0=dmax[c][:, :], scalar=-2.0,
                    in1=s_all[c][:, :], op0=Alu.mult, op1=Alu.add)

            nc.sync.dma_start(out=out_d[:, :], in_=loss_t[:, :])
    nc.compile()
    return nc


def kernel(y_true: np.ndarray, y_pred: np.ndarray) -> np.ndarray:
    from concourse.bass_utils import run_bass_kernel_spmd

    if "nc" not in _CACHE:
        _CACHE["nc"] = _build()
    nc = _CACHE["nc"]

    yt = np.asarray(y_true, dtype=np.float32).reshape(B, N * D).astype(np.float16)
    yp = np.asarray(y_pred, dtype=np.float32).reshape(B, N * D).astype(np.float16)

    in_maps = [
        {
            "yt": np.ascontiguousarray(yt[c * B_LOC:(c + 1) * B_LOC]),
            "yp": np.ascontiguousarray(yp[c * B_LOC:(c + 1) * B_LOC]),
        }
        for c in range(N_CORES)
    ]
    res = run_bass_kernel_spmd(nc, in_maps, list(range(N_CORES)), trace=TRACE)
    _CACHE["last_results"] = res
    vals = np.concatenate([np.asarray(r["out"], dtype=np.float64).reshape(-1)
                           for r in res.results])
    loss = vals.mean() / (D * N)
    return np.float32(loss)
